# revision 6
# baseline (speedup 1.0000x reference)
"""Trainium2 Bass kernel for CTC loss — blocked-scan column formulation.

Strategy
--------
Pure data parallel across 8 NeuronCores: 32 batch elements per core.

CTC forward in linear space, reformulated per label column (E_k = blank state
2k, O_k = label state 2k+1):

    e_t = (e_{t-1} + o^{k-1}_{t-1}) * pb_t          (pb = blank prob)
    o_t = (o_{t-1} + e_{t-1} + m_k o^{k-1}_{t-1}) * pl_t[k]

Both are first-order recurrences along t -> one `tensor_tensor_scan`
(op0=add, op1=mult) each.  The DP becomes, per time-block of length G:
a sequential loop over 65 columns x 3 big DVE ops (scanE, stt, scanO)
instead of 512 steps x 8 small ops.  Time is split into blocks
SCHED=(128,256,128) with a carry renorm (K/max recentring) at block
boundaries; numpy-validated fp32-exact at rel_err 1.1e-3 incl. bf16
staging of the gathered p series.

Phases per core:
  A (streamed): per (ti, 4-batch group) DMA y_pred -> GPSIMD ap_gather of
     [blank, y_0..y_63] columns -> +EPS & bf16 cast on ACT -> DMA to DRAM
     rbuf[b, t, 66].  y_pred loads issue from the ACT sequencer to keep the
     SP sequencer free for the write/chunk stream.
  B (DP): per block, one chunk DMA [32b, G, 66] bf16, then the 65-column
     scan loop on DVE; carries renormed into t_car.
  C: ll = ln(Ecar_64 + Ocar_63) + sum_j ln(mx_j / K); out = -ll.
"""
import numpy as np
import concourse.bacc as bacc
import concourse.tile as tile
from concourse import mybir
from concourse.bass_utils import run_bass_kernel_spmd

F32 = mybir.dt.float32
BF16 = mybir.dt.bfloat16
I16 = mybir.dt.int16
ALU = mybir.AluOpType
ACTF = mybir.ActivationFunctionType
AXL = mybir.AxisListType

B, T, C, L = 256, 512, 512, 64
NCORES = 8
BC = B // NCORES         # 32 batch elements per core
BLANK = C - 1
EPS = 1e-7
NI = 80                  # gather width: [blank, y_0..63, 15 pad] (mult of 16)
NW = NI // 16            # 5 wrapped idx columns
NWP = 6                  # padded idx stride (12B, 4B aligned)
NC_ROW = 66              # rbuf row: [pb, pl_0..63, pad] (132B, 4B aligned)
SCHED = (128, 256, 128)  # t-block lengths; renorm at boundaries (validated)
NB = len(SCHED)
GMAX = max(SCHED)
K_SCALE = float(2.0 ** 64)
K_INV = float(2.0 ** -64)

_NC_CACHE = None
DEBUG = False


def _build():
    nc = bacc.Bacc("TRN2", target_bir_lowering=False, debug=False)
    d_yp = nc.dram_tensor("yp", [BC, T, C], F32, kind="ExternalInput")
    d_gidx = nc.dram_tensor("gidx", [128, BC * NWP], I16, kind="ExternalInput")
    d_m = nc.dram_tensor("m", [BC, L], F32, kind="ExternalInput")
    d_out = nc.dram_tensor("out", [BC, 1], F32, kind="ExternalOutput")
    d_dbg = nc.dram_tensor("dbg", [NB, BC, 2 * L + 1], F32, kind="ExternalOutput") if DEBUG else None
    d_dbg2 = nc.dram_tensor("dbg2", [BC, 8], F32, kind="ExternalOutput") if DEBUG else None

    with tile.TileContext(nc) as tc, \
         tc.tile_pool(name="const", bufs=1) as constp, \
         tc.tile_pool(name="ypp", bufs=3) as ypp, \
         tc.tile_pool(name="gp", bufs=3) as gp, \
         tc.tile_pool(name="gbp", bufs=3) as gbp, \
         tc.tile_pool(name="pcp", bufs=1) as pcp, \
         tc.tile_pool(name="dpp", bufs=1) as dpp, \
         tc.tile_pool(name="dramp", bufs=1, space="DRAM") as dramp:

        t_gidx = constp.tile([128, BC * NWP], I16, tag="gidx")
        nc.sync.dma_start(t_gidx[:], d_gidx[:])
        t_m = constp.tile([BC, L], F32, tag="m")
        nc.sync.dma_start(t_m[:], d_m[:])

        rbuf = dramp.tile([BC, T, NC_ROW], BF16, tag="rbuf", name="rbuf")

        # ---- DP state tiles ----
        t_O = dpp.tile([BC, L, GMAX + 1], F32, tag="O")      # odd series, slot0=carry
        t_E = dpp.tile([BC, GMAX + 1], F32, tag="E")         # current E series
        t_c = dpp.tile([BC, GMAX], F32, tag="c")             # stt temp
        t_z = dpp.tile([BC, GMAX], F32, tag="z")             # zeros (k=0 scanE data0)
        t_car = dpp.tile([BC, 2 * L + 1], F32, tag="car")    # E car 0..64 | O car 65..128
        t_led = dpp.tile([BC, NB], F32, tag="led")           # raw block maxes
        t_rcp = dpp.tile([BC, 1], F32, tag="rcp")
        t_rs = dpp.tile([BC, 1], F32, tag="rs")

        nc.vector.memset(t_z[:], 0.0)
        nc.vector.memset(t_car[:], 0.0)
        nc.vector.memset(t_E[:, 0:1], 0.0)
        nc.vector.memset(t_O[:, :, 0:1], 0.0)

        t_p0 = pcp.tile([BC, SCHED[0], NC_ROW], BF16, tag="pch0", name="pch0")

        emitted_ti = 0
        t0 = 0
        for j, G in enumerate(SCHED):
            # ---- Phase A for the time range this block needs ----
            need_ti = (t0 + G + 127) // 128
            for ti in range(emitted_ti, need_ti):
                for bg in range(BC // 4):
                    # 4 batch elements per DMA/cast/write: keeps the ACT/SP
                    # sequencers unclogged (Pool ap_gather is the phase-A
                    # critical engine at ~780ns per call)
                    t_yp = ypp.tile([128, 4, C], F32, tag="yp")
                    src = d_yp[4 * bg:4 * bg + 4, ti * 128:(ti + 1) * 128, :]
                    nc.scalar.dma_start(t_yp[:], src.transpose([1, 0, 2]))
                    t_g = gp.tile([128, 4, NI], F32, tag="g")
                    for bi in range(4):
                        b = 4 * bg + bi
                        nc.gpsimd.ap_gather(
                            t_g[:, bi, :], t_yp[:, bi, :],
                            t_gidx[:, b * NWP:b * NWP + NW],
                            channels=128, num_elems=C, d=1, num_idxs=NI,
                        )
                    t_gb = gbp.tile([128, 4, NC_ROW], BF16, tag="gb")
                    # +EPS and bf16 downcast on ACT
                    nc.scalar.activation(t_gb[:], t_g[:, :, 0:NC_ROW], ACTF.Copy, bias=EPS)
                    dst = rbuf[4 * bg:4 * bg + 4, ti * 128:(ti + 1) * 128, :]
                    nc.sync.dma_start(dst.transpose([1, 0, 2]), t_gb[:])
            emitted_ti = need_ti

            # ---- Phase B: DP block ----
            # one wide [32, ...] load: DMA cost is per-partition-line bytes,
            # so narrow/partial slices would multiply pipe time
            if j == 0:
                t_p = t_p0
            else:
                t_p = pcp.tile([BC, G, NC_ROW], BF16, tag=f"pch{j}", name=f"pch{j}")
            nc.sync.dma_start(t_p[:], rbuf[:, t0:t0 + G, :])
            pb = t_p[:, 0:G, 0]

            if j > 0:
                # place rescaled O carries into series slot0 (shifted reads)
                nc.vector.tensor_copy(t_O[:, :, 0], t_car[:, L + 1:2 * L + 1])

            for k in range(L + 1):
                first = (j == 0)
                initE = 1.0 if (first and k == 0) else t_car[:, k:k + 1]
                if j > 0 and k < L:
                    # E slot0 = E_k carry-in (slot0 of the shifted e-series
                    # read by stt/scanO); stays 0 throughout block 0
                    nc.vector.tensor_copy(t_E[:, 0:1], t_car[:, k:k + 1])
                d0E = t_z[:, 0:G] if k == 0 else t_O[:, k - 1, 0:G]
                nc.vector.tensor_tensor_scan(
                    t_E[:, 1:G + 1], d0E, pb, initE, ALU.add, ALU.mult)
                nc.vector.tensor_copy(t_car[:, k:k + 1], t_E[:, G:G + 1])
                if k == L:
                    break
                pl = t_p[:, 0:G, 1 + k]
                initO = 1.0 if (first and k == 0) else t_car[:, L + 1 + k:L + 2 + k]
                if k == 0:
                    d0O = t_E[:, 0:G]
                else:
                    nc.vector.scalar_tensor_tensor(
                        t_c[:, 0:G], t_O[:, k - 1, 0:G], t_m[:, k:k + 1],
                        t_E[:, 0:G], op0=ALU.mult, op1=ALU.add)
                    # O-carry writeback of the previous column doubles as a
                    # filler in the stt->scanO sem-propagation gap
                    nc.vector.tensor_copy(
                        t_car[:, L + k:L + k + 1], t_O[:, k - 1, G:G + 1])
                    d0O = t_c[:, 0:G]
                nc.vector.tensor_tensor_scan(
                    t_O[:, k, 1:G + 1], d0O, pl, initO, ALU.add, ALU.mult)

            # ---- block end: last O carry, renorm ----
            nc.vector.tensor_copy(
                t_car[:, 2 * L:2 * L + 1], t_O[:, L - 1, G:G + 1])
            nc.vector.tensor_reduce(t_led[:, j:j + 1], t_car[:], AXL.X, ALU.max)
            nc.vector.reciprocal(t_rcp[:], t_led[:, j:j + 1])
            nc.vector.tensor_scalar_mul(t_rs[:], t_rcp[:], K_SCALE)
            nc.vector.tensor_scalar_mul(t_car[:], t_car[:], t_rs[:, 0:1])
            if DEBUG:
                nc.sync.dma_start(d_dbg[j], t_car[:])
            t0 += G

        # ---- Phase C: finalize ----
        # ACT Ln is table-accurate only for args in [2^-64, 2^64]; the final
        # sum sits at ~K=2^64 so scale it down by K_INV (exact lnK correction
        # folded into CONST); raw block maxes are already in-range.
        t_f0 = dpp.tile([BC, 1], F32, tag="f0")
        nc.vector.tensor_add(t_f0[:], t_car[:, L:L + 1], t_car[:, 2 * L:2 * L + 1])
        t_vals = dpp.tile([BC, NB + 1], F32, tag="vals")
        nc.scalar.activation(t_vals[:, 0:1], t_f0[:], ACTF.Ln, scale=K_INV)
        nc.scalar.activation(t_vals[:, 1:NB + 1], t_led[:], ACTF.Ln)
        t_rsum = dpp.tile([BC, 1], F32, tag="rsum")
        nc.vector.tensor_reduce(t_rsum[:], t_vals[:], AXL.X, ALU.add)
        t_res = dpp.tile([BC, 1], F32, tag="res")
        # ll = (ln(sum)-lnK) + sum_j ln(mx_j) - NB*lnK + lnK  => const = -(NB-1)*lnK
        CONST = -(NB - 1) * float(np.log(K_SCALE))
        nc.vector.tensor_scalar(
            t_res[:], t_rsum[:], CONST, -1.0, op0=ALU.add, op1=ALU.mult)
        nc.sync.dma_start(d_out[:], t_res[:])
        if DEBUG:
            t_d2 = dpp.tile([BC, 8], F32, tag="d2")
            nc.vector.tensor_copy(t_d2[:, 0:NB], t_led[:])
            nc.vector.tensor_copy(t_d2[:, 3:4], t_f0[:])
            nc.vector.tensor_copy(t_d2[:, 4:5], t_fl[:])
            nc.vector.tensor_copy(t_d2[:, 5:5 + NB], t_rl[:])
            nc.sync.dma_start(d_dbg2[:], t_d2[:])

    nc.compile()
    return nc


def _host_prep(y_true, y_pred):
    y_true = np.asarray(y_true)
    y_pred = np.asarray(y_pred, dtype=np.float32)
    assert y_true.shape == (B, L), y_true.shape
    assert y_pred.shape == (B, T, C), y_pred.shape

    idx = np.zeros((B, NI), np.int16)
    idx[:, 0] = BLANK
    idx[:, 1:1 + L] = y_true.astype(np.int16)
    w = idx.reshape(B, NW, 16)

    m = np.zeros((B, L), np.float32)
    m[:, 1:] = (y_true[:, 1:] != y_true[:, :-1]).astype(np.float32)

    in_maps = []
    for cc in range(NCORES):
        sl = slice(cc * BC, (cc + 1) * BC)
        wc = w[sl]                                          # [BC, NW, 16]
        gidx5 = np.tile(wc.transpose(2, 0, 1), (8, 1, 1))   # [128, BC, NW]
        gidx = np.zeros((128, BC, NWP), np.int16)
        gidx[:, :, :NW] = gidx5
        in_maps.append({
            "yp": np.ascontiguousarray(y_pred[sl]),
            "gidx": np.ascontiguousarray(gidx.reshape(128, BC * NWP)),
            "m": np.ascontiguousarray(m[sl]),
        })
    return in_maps


def kernel(y_true, y_pred):
    global _NC_CACHE
    in_maps = _host_prep(y_true, y_pred)
    if _NC_CACHE is None:
        _NC_CACHE = _build()
    res = run_bass_kernel_spmd(_NC_CACHE, in_maps, core_ids=list(range(NCORES)))
    out = np.concatenate([res.results[cc]["out"] for cc in range(NCORES)], axis=0)
    return np.ascontiguousarray(out.astype(np.float32))
